# revision 2
# baseline (speedup 1.0000x reference)
# Trainium2 Bass kernel for KNN-style sparse cross-attention — v2.
#
# Reformulation that never materializes k or v:
#   logits[q,h,t] = <W_k[h]^T q[q,h-chunk], tgt[q,t,:]>   (g-vectors vs RAW tgt)
#   u[q,h,:]      = sum_t attn[q,h,t] * tgt[q,t,:]        (aggregate RAW tgt)
#   out           = W_o @ concat_h(W_v[h] u[q,h,:])
# This removes the [RT x D x 2D] kv projection (4.3 GMAC/core): ~0.34 GMAC/core
# of small matmuls remain.
#
# Per core: R=256 queries x T=32 keys, processed in 16 groups of 16 queries.
# Group pipeline (software-pipelined so the PE never waits on softmax):
#   logits = 4 accum-matmuls (stationary = 128 g-columns (8h x 16q), moving =
#   tgtT [128 x 512]) + a 5th P=18 matmul that adds the block-diag -30k mask
#   and padding bias (16 rank-1 diag selectors + const row + pad row);
#   exp with accum_out row-sum -> normalize on ACT -> 4 PE transposes flip
#   attn to (q,t)-partitions -> 4x4 accum-matmuls against raw tgt rows
#   aggregate u -> W_v / W_o once at the end.
# All DRAM buffers are host-packed so every DMA moves 4-16KB per partition row.
import os
from contextlib import ExitStack

import numpy as np

import concourse.bacc as bacc
import concourse.mybir as mybir
import concourse.tile as tile
from concourse import bass_utils
from concourse.masks import make_identity

N_CORES = 8
D = 512          # d_model
H = 8            # heads
DH = 64          # head dim
T = 32           # KNN set size per query
BS = 2048        # B*S total queries
R = BS // N_CORES     # queries per core (256)
RT = R * T            # kv rows per core (8192)
PT = 128              # partition tile
KD = D // PT          # 4 tiles over d_model
GQ = 16               # queries per group (GQ*H = 128 partitions)
NG = R // GQ          # 16 groups
GW = GQ * T           # 512 kv columns per group
SQ = 4                # queries per slab (SQ*T = 128 partitions)
NS = GQ // SQ         # 4 slabs per group
PM = GQ + 2           # mask-matmul contraction rows (16 diag + const + pad)

F32 = mybir.dt.float32
F16 = mybir.dt.float16
F8 = mybir.dt.float8e4
ACTF = mybir.ActivationFunctionType

MASK_VAL = -30000.0
FP8_A = os.environ.get("V2_FP8A", "0") == "1"   # logits side (tgtT + g-vectors)
FP8_R = os.environ.get("V2_FP8R", "0") == "1"   # aggregate side (tgt rows + attn)


def build_program(n_cores=N_CORES):
    adt = F16
    gdt = F8 if FP8_A else F16
    rdt = F8 if FP8_R else F16

    nc = bacc.Bacc(
        "TRN2",
        target_bir_lowering=False,
        debug=False,
        enable_asserts=False,
        num_devices=n_cores,
    )

    # All host-packed: partition-row-major with fat rows.
    srcP = nc.dram_tensor("srcP", [PT, KD * R], F16, kind="ExternalInput").ap()
    tgtA = nc.dram_tensor("tgtA", [NG * PT, KD * GW], gdt, kind="ExternalInput").ap()
    tgtR = nc.dram_tensor("tgtR", [NG * PT, NS * D], rdt, kind="ExternalInput").ap()
    biasP = nc.dram_tensor("biasP", [PT, NG * GW], F16, kind="ExternalInput").ap()
    zmask = nc.dram_tensor("zmask", [PT, R], F32, kind="ExternalInput").ap()
    wsP = nc.dram_tensor("wsP", [PT, KD * D], F16, kind="ExternalInput").ap()
    wkP = nc.dram_tensor("wkP", [PT, KD * D], F16, kind="ExternalInput").ap()
    wvP = nc.dram_tensor("wvP", [PT, KD * D], F16, kind="ExternalInput").ap()
    woP = nc.dram_tensor("woP", [PT, KD * D], F16, kind="ExternalInput").ap()
    outT = nc.dram_tensor("outT", [D, R], F32, kind="ExternalOutput").ap()

    lp = nc.allow_low_precision("fp32 PSUM accumulate, 16-bit activations")
    lp.__enter__()
    with tile.TileContext(nc) as tc, ExitStack() as ctx:
        consts = ctx.enter_context(tc.tile_pool(name="consts", bufs=1))
        io = ctx.enter_context(tc.tile_pool(name="io", bufs=2))
        work = ctx.enter_context(tc.tile_pool(name="work", bufs=2))
        ps_l = ctx.enter_context(tc.tile_pool(name="ps_l", bufs=2, space="PSUM"))
        ps_u = ctx.enter_context(tc.tile_pool(name="ps_u", bufs=2, space="PSUM"))
        ps_t = ctx.enter_context(tc.tile_pool(name="ps_t", bufs=1, space="PSUM"))
        ps_c = ctx.enter_context(tc.tile_pool(name="ps_c", bufs=3, space="PSUM"))

        # ---- weights / constants first so the prologue compute starts early ----
        ws_sb = consts.tile([PT, KD * D], F16, name="ws_sb")
        nc.sync.dma_start(ws_sb, wsP)
        wk_sb = consts.tile([PT, KD * D], F16, name="wk_sb")
        nc.sync.dma_start(wk_sb, wkP)
        src_sb = consts.tile([PT, KD * R], F16, name="src_sb")
        nc.sync.dma_start(src_sb, srcP)
        bs_sb = consts.tile([PT, NG * GW], F16, name="bs_sb")
        nc.sync.dma_start(bs_sb, biasP)
        wv_sb = consts.tile([PT, KD * D], F16, name="wv_sb")
        nc.sync.dma_start(wv_sb, wvP)
        wo_sb = consts.tile([PT, KD * D], F16, name="wo_sb")
        nc.sync.dma_start(wo_sb, woP)
        zm_sb = consts.tile([PT, R], F32, name="zm_sb")
        nc.sync.dma_start(zm_sb, zmask)
        eye_sb = consts.tile([PT, PT], F16, name="eye_sb")
        make_identity(nc, eye_sb)

        # ---- HAM warm-up: keep the PE busy while weights stream in, so the
        # clock gate is released (1.2 -> 2.4 GHz) before real work starts ----
        wup = ps_c.tile([PT, R], F32, name="wup", tag="c")
        for _ in range(36):
            nc.tensor.matmul(wup[:, :DH], eye_sb, eye_sb[:, :DH], start=True, stop=True)

        # ---- q projection: qT[d%128, (m, q)] ----
        qT = consts.tile([PT, KD * R], adt, name="qT")
        for m in range(KD):
            qp = ps_c.tile([PT, R], F32, name="qp", tag="c")
            for j in range(KD):
                nc.tensor.matmul(
                    qp,
                    ws_sb[:, j * D + m * PT : j * D + (m + 1) * PT],
                    src_sb[:, j * R : (j + 1) * R],
                    start=(j == 0),
                    stop=(j == KD - 1),
                )
            nc.scalar.copy(qT[:, m * R : (m + 1) * R], qp)

        # ---- g-vectors: GT[d%128, (dt, g, h*16+q)] = W_k[h]^T q_h ----
        GT = consts.tile([PT, KD * H * R], gdt, name="GT")
        GTv = GT.rearrange("p (a g h q) -> p a g h q", a=KD, g=NG, h=H)
        for dt in range(KD):
            for h in range(H):
                po = (h % 2) * DH
                gp = ps_c.tile([PT, R], F32, name="gp", tag="c")
                nc.tensor.matmul(
                    gp,
                    wk_sb[po : po + DH, (h // 2) * D + dt * PT : (h // 2) * D + (dt + 1) * PT],
                    qT[po : po + DH, (h // 2) * R : (h // 2 + 1) * R],
                    start=True,
                    stop=True,
                )
                gv = gp.rearrange("p (g q) -> p g q", g=NG)
                if h % 2 == 0:
                    nc.scalar.copy(GTv[:, dt, :, h, :], gv)
                else:
                    nc.vector.tensor_copy(GTv[:, dt, :, h, :], gv)

        # ---- uT accumulator: [d%128, (dt, h, q_global)] ----
        uT = consts.tile([PT, KD * H * R], adt, name="uT")
        uTv = uT.rearrange("p (a h q) -> p a h q", a=KD, h=H)

        avT = consts.tile([PT, KD * R], adt, name="avT")

        def epilogue_half(hq):
            """W_v / W_o projections for queries [hq*128, (hq+1)*128)."""
            q0 = hq * PT
            for h in range(H):
                ap_ = ps_c.tile([PT, R], F32, name="ap_", tag="c")
                apo = ap_[:DH, :PT]
                for dt in range(KD):
                    nc.tensor.matmul(
                        apo,
                        wv_sb[:, dt * D + h * DH : dt * D + (h + 1) * DH],
                        uT[:, dt * R * H + h * R + q0 : dt * R * H + h * R + q0 + PT],
                        start=(dt == 0),
                        stop=(dt == KD - 1),
                    )
                po = (h % 2) * DH
                nc.scalar.copy(
                    avT[po : po + DH, (h // 2) * R + q0 : (h // 2) * R + q0 + PT], apo
                )
            for e in range(KD):
                op = ps_c.tile([PT, R], F32, name="op", tag="c")
                opo = op[:, :PT]
                for dt in range(KD):
                    nc.tensor.matmul(
                        opo,
                        wo_sb[:, dt * D + e * PT : dt * D + (e + 1) * PT],
                        avT[:, dt * R + q0 : dt * R + q0 + PT],
                        start=(dt == 0),
                        stop=(dt == KD - 1),
                    )
                res = work.tile([PT, PT], F32, name="res")
                nc.vector.tensor_mul(res, opo, zm_sb[:, q0 : q0 + PT])
                nc.sync.dma_start(outT[e * PT : (e + 1) * PT, q0 : q0 + PT], res)

        # ---- 3-stage software-pipelined group loop: logits/softmax for group
        # g, transposes for g-1, aggregation for g-2 — the PE instruction
        # stream never waits on the softmax or the ts copy. ----
        stA, stT = {}, {}
        for it in range(NG + 2):
            newA = {}
            if it < NG:
                g = it
                tgA = io.tile([PT, KD * GW], gdt, name="tgA")
                nc.sync.dma_start(tgA, tgtA[g * PT : (g + 1) * PT, :])
                tgR = io.tile([PT, NS * D], rdt, name="tgR", bufs=4)
                nc.sync.dma_start(tgR, tgtR[g * PT : (g + 1) * PT, :])

                # logits: [128=(8h,16q) x 512=(16q,32t)]
                pg = ps_l.tile([PT, GW], F32, name="pg")
                for dt in range(KD):
                    nc.tensor.matmul(
                        pg,
                        GT[:, dt * H * R + g * PT : dt * H * R + (g + 1) * PT],
                        tgA[:, dt * GW : (dt + 1) * GW],
                        start=(dt == 0),
                        stop=(dt == KD - 1),
                    )
                nc.vector.tensor_add(pg, pg, bs_sb[:, g * GW : (g + 1) * GW])
                en = work.tile([PT, GW], adt, name="en")
                dn = work.tile([PT, 1], F32, name="dn")
                nc.scalar.activation(en, pg, ACTF.Exp, accum_out=dn)
                rec = work.tile([PT, 1], F32, name="rec")
                nc.vector.reciprocal(rec, dn)
                nc.vector.tensor_mul(en, en, rec.broadcast_to([PT, GW]))
                newA = dict(g=g, en=en, tgR=tgR)

            if stA:
                # stage T: transpose slabs -> ts[(4q,32t) x (8h,16q)]
                en_ = stA["en"]
                ts = work.tile([PT, NS * PT], rdt, name="ts")
                tp = ps_t.tile([PT, NS * PT], adt, name="tsp")
                for s in range(NS):
                    nc.tensor.transpose(
                        tp[:, s * PT : (s + 1) * PT],
                        en_[:, s * PT : (s + 1) * PT],
                        eye_sb,
                    )
                nc.vector.tensor_copy(ts, tp)
                stA["ts"] = ts

            if stT:
                # stage U: u-aggregation; each slab's ts block is zero outside
                # its own 4 queries, so accumulating all 4 slabs over the full
                # 128 (h,q) columns is exact.
                gu, tsu, tgRu = stT["g"], stT["ts"], stT["tgR"]
                up = ps_u.tile([PT, KD * PT], F32, name="up")
                upv = up.rearrange("p (a h q) -> p a h q", a=KD, h=H)
                for dt in range(KD):
                    for s in range(NS):
                        nc.tensor.matmul(
                            up[:, dt * PT : (dt + 1) * PT],
                            tgRu[:, s * D + dt * PT : s * D + (dt + 1) * PT],
                            tsu[:, s * PT : (s + 1) * PT],
                            start=(s == 0),
                            stop=(s == NS - 1),
                        )
                for dt in range(KD):
                    dst = uTv[:, dt, :, gu * GQ : (gu + 1) * GQ]
                    if dt % 2 == 0:
                        nc.scalar.copy(dst, upv[:, dt])
                    else:
                        nc.vector.tensor_copy(dst, upv[:, dt])

            stT, stA = stA, newA
            if it == NG // 2 + 1:
                # first-half queries fully aggregated: overlap their V/O
                # projections with the remaining groups
                epilogue_half(0)
        epilogue_half(1)

    lp.__exit__(None, None, None)
    nc.compile()
    return nc


_PROGRAM = None


def _get_program():
    global _PROGRAM
    if _PROGRAM is None:
        _PROGRAM = build_program()
    return _PROGRAM


def _pack_rows(w):
    """[D, M] -> [128, KD*M] with row p holding all KD chunks (fat DMA rows)."""
    Dd, M = w.shape
    return np.ascontiguousarray(
        w.reshape(KD, PT, M).transpose(1, 0, 2).reshape(PT, KD * M)
    )


def prep_inputs(src, tgt, tgt_padding_mask, in_proj_weight, in_proj_bias,
                out_proj_weight, out_proj_bias):
    f32 = np.float32
    f16 = np.float16
    src2 = np.asarray(src, dtype=f32).reshape(BS, D)
    tgt2 = np.asarray(tgt, dtype=f32).reshape(BS * T, D)
    mask2 = np.asarray(tgt_padding_mask).astype(bool).reshape(BS, T)
    wm = np.asarray(in_proj_weight, dtype=f32)
    wo = np.asarray(out_proj_weight, dtype=f32)

    wsP = _pack_rows(((wm[:D] / np.sqrt(DH)).T).astype(f16))
    wkP = _pack_rows(wm[D : 2 * D].astype(f16))
    wvP = _pack_rows((wm[2 * D :].T).astype(f16))
    woP = _pack_rows((wo.T).astype(f16))

    # partition p = h*16 + q(within group); col = q'*32 + t
    hq_q = np.arange(PT) % GQ                # row -> q (within group)
    cq = np.arange(GW) // T                  # col -> q' (within group)
    diag = np.where(hq_q[:, None] == cq[None, :], 0.0, MASK_VAL)  # [128, 512]

    in_maps = []
    for c in range(N_CORES):
        rows = slice(c * R, (c + 1) * R)
        kvrows = slice(c * RT, (c + 1) * RT)
        mask_c = mask2[rows]
        novalid = mask_c.all(axis=-1)
        invalid = mask_c & ~novalid[:, None]
        padv = np.where(invalid, f32(MASK_VAL), 0.0)          # [R, T]
        padc = padv.reshape(NG, GW)                           # [NG, 512]
        bias = diag[None, :, :] + padc[:, None, :]            # [NG, 128, 512]
        bias = np.maximum(bias, 2 * MASK_VAL)
        biasP = np.ascontiguousarray(
            bias.transpose(1, 0, 2).reshape(PT, NG * GW)
        ).astype(f16)

        import ml_dtypes
        adt_np = ml_dtypes.float8_e4m3fn if FP8_A else f16
        rdt_np = ml_dtypes.float8_e4m3fn if FP8_R else f16
        srcTc = src2[rows].T.astype(f16)                      # [D, R]
        tgtTc = tgt2[kvrows].T                                # [D, RT]
        tgtAc = np.ascontiguousarray(
            tgtTc.reshape(KD, PT, NG, GW).transpose(2, 1, 0, 3).reshape(NG * PT, KD * GW)
        ).astype(adt_np)
        tgtRc = np.ascontiguousarray(
            tgt2[kvrows].reshape(NG, NS, PT, D).transpose(0, 2, 1, 3).reshape(NG * PT, NS * D)
        ).astype(rdt_np)
        in_maps.append({
            "srcP": _pack_rows(srcTc),
            "tgtA": tgtAc,
            "tgtR": tgtRc,
            "biasP": biasP,
            "zmask": np.ascontiguousarray(
                np.broadcast_to((~novalid).astype(f32), (PT, R))
            ),
            "wsP": wsP, "wkP": wkP, "wvP": wvP, "woP": woP,
        })
    return in_maps


def _numpy_fallback(src, tgt, tgt_padding_mask, in_proj_weight, in_proj_bias,
                    out_proj_weight, out_proj_bias):
    B, S, _ = src.shape
    w_src, w_tgt = in_proj_weight[:D], in_proj_weight[D:]
    b_src, b_tgt = in_proj_bias[:D], in_proj_bias[D:]
    q = src @ w_src.T + b_src
    kv = tgt @ w_tgt.T + b_tgt
    k, v = kv[..., :D], kv[..., D:]
    inv = tgt_padding_mask.astype(bool)
    noval = inv.all(-1)
    inv = inv & ~noval[..., None]
    q = q.reshape(B, S, H, DH)
    k = k.reshape(B, S, T, H, DH)
    v = v.reshape(B, S, T, H, DH)
    att = np.einsum("bshd,bsthd->bhst", q, k)
    att = np.where(inv[:, None], -np.inf, att) / np.sqrt(DH)
    att = att - att.max(-1, keepdims=True)
    att = np.exp(att)
    att = att / att.sum(-1, keepdims=True)
    out = np.einsum("bhst,bsthd->bshd", att, v).reshape(B, S, D)
    out = out @ out_proj_weight.T + out_proj_bias
    return np.where(noval[..., None], 0.0, out).astype(np.float32)


def run(inputs, trace=False):
    in_maps = prep_inputs(**inputs)
    nc = _get_program()
    res = bass_utils.run_bass_kernel_spmd(
        nc, in_maps, core_ids=list(range(N_CORES)), trace=trace
    )
    out = np.empty((BS, D), dtype=np.float32)
    for c in range(N_CORES):
        out[c * R : (c + 1) * R] = res.results[c]["outT"].T
    return out.reshape(4, 512, D), res


def kernel(**inputs):
    inputs = {k: np.asarray(v) for k, v in inputs.items()}
    if (np.any(inputs["in_proj_bias"]) or np.any(inputs["out_proj_bias"])):
        return _numpy_fallback(**inputs)
    out, _ = run(inputs)
    return out
